# revision 1
# baseline (speedup 1.0000x reference)
"""Trainium2 Bass kernel for nn_LogicalOperatorFusion.

Semantics (matches the jax reference):
  fv = field_vectors                                  [B=1024, NF=64, H=512] f32
  fv[:, not_idx] = tanh(fv[:, not_idx] @ not_W + not_b)
  cat_and = concat(fv[:, and_pairs[:,0]], fv[:, and_pairs[:,1]], -1)   [B,16,1024]
  fused_and = relu(cat_and @ and_W1 + and_b1) @ and_W2 + and_b2        [B,16,512]
  cat_or  = ... same with or_pairs (8 pairs) ...                       [B,8,512]
  out = concat([fused_and, fused_or, fv[:, leftover_idx]], axis=1)     [B,40,512]

Strategy:
  - Data parallel: batch sharded 8 ways (128 rows/core), weights replicated.
  - Host pre-transposes activations so the PE stationary operand (lhsT =
    X^T chunks, [K=128 h, M=128 rows]) arrives DMA-ready; weights are the
    N=512 moving operand.  Only the hidden layer h needs on-chip transposes
    (4 PE identity-transposes per pair).
  - Biases are applied as rank-1 (K=1) ones (x) b matmuls accumulated in PSUM,
    so ACT does single-pass relu/tanh PSUM->SBUF.
  - Untouched leftover fields (not in not_idx) never touch the device; the
    host copies them straight from the input.
"""

import sys

for p in ("/opt/trn_rl_repo",):
    if p not in sys.path:
        sys.path.insert(0, p)

import numpy as np
import ml_dtypes

BF16_NP = ml_dtypes.bfloat16

import concourse.bacc as bacc
import concourse.mybir as mybir
from concourse.bass_utils import run_bass_kernel_spmd
from concourse.tile import TileContext

B, NF, H = 1024, 64, 512
NCORES = 8
BS = B // NCORES  # 128 rows per core
NAND, NOR, NNOT = 16, 8, 8
NPAIR = NAND + NOR  # 24
NOUT = NPAIR + NNOT  # 32 device output slots
KC1 = (2 * H) // 128  # 8 k-chunks for layer 1
KC2 = H // 128  # 4 k-chunks for layer 2 / not
F32 = mybir.dt.float32
BF16 = mybir.dt.bfloat16

TRACE = False  # test.py sets this for profiled runs
LAST_RESULT = None  # BassKernelResults of the last run (for test.py)

_NC = None  # cached traced Bass program


def _build_nc():
    """Trace the per-core Bass program (identical on all 8 cores)."""
    nc = bacc.Bacc("TRN2", target_bir_lowering=False)

    pairs_in = nc.dram_tensor("pairs_in", [NPAIR, 128, 2 * H], BF16, kind="ExternalInput")
    nots_in = nc.dram_tensor("nots_in", [NNOT, 128, H], BF16, kind="ExternalInput")
    and_w1t = nc.dram_tensor("and_w1t", [128, KC1 * H], BF16, kind="ExternalInput")
    or_w1t = nc.dram_tensor("or_w1t", [128, KC1 * H], BF16, kind="ExternalInput")
    and_w2t = nc.dram_tensor("and_w2t", [128, KC2 * H], BF16, kind="ExternalInput")
    or_w2t = nc.dram_tensor("or_w2t", [128, KC2 * H], BF16, kind="ExternalInput")
    not_wt = nc.dram_tensor("not_wt", [128, KC2 * H], BF16, kind="ExternalInput")
    biases_in = nc.dram_tensor("biases", [5, 1, H], BF16, kind="ExternalInput")
    b2bc_in = nc.dram_tensor("b2bc", [2, 128, H], F32, kind="ExternalInput")
    b1bc_in = nc.dram_tensor("b1bc", [2, 128, H], F32, kind="ExternalInput")
    ident_in = nc.dram_tensor("ident", [128, 128], BF16, kind="ExternalInput")
    out_d = nc.dram_tensor("out", [NOUT, 128, H], F32, kind="ExternalOutput")

    with TileContext(nc) as tc:
        with (
            tc.tile_pool(name="consts", bufs=1) as consts,
            tc.tile_pool(name="weights", bufs=1) as wpool,
            tc.tile_pool(name="pairin", bufs=4) as inpool,
            tc.tile_pool(name="notin", bufs=3) as notpool,
            tc.tile_pool(name="hid", bufs=3) as hpool,
            tc.tile_pool(name="hidT", bufs=3) as htpool,
            tc.tile_pool(name="outs", bufs=4) as opool,
            tc.tile_pool(name="psum", bufs=2, space="PSUM") as psum,
        ):
            ident = consts.tile([128, 128], BF16, tag="ident")
            nc.sync.dma_start(out=ident, in_=ident_in[:, :])
            ones = consts.tile([1, 128], BF16, tag="ones")
            nc.vector.memset(ones, 1.0)
            bt = []
            for i in range(5):  # and_b1, or_b1, and_b2, or_b2, not_b
                b = consts.tile([1, H], BF16, tag=f"bias{i}")
                nc.sync.dma_start(out=b, in_=biases_in[i, :, :])
                bt.append(b)
            b_not = bt[4]
            b2bc_and = consts.tile([128, H], F32, tag="b2bca")
            nc.sync.dma_start(out=b2bc_and, in_=b2bc_in[0, :, :])
            b2bc_or = consts.tile([128, H], F32, tag="b2bco")
            nc.sync.dma_start(out=b2bc_or, in_=b2bc_in[1, :, :])
            b2bc = {True: b2bc_and, False: b2bc_or}
            b1bc_and = consts.tile([128, H], F32, tag="b1bca")
            nc.sync.dma_start(out=b1bc_and, in_=b1bc_in[0, :, :])
            b1bc_or = consts.tile([128, H], F32, tag="b1bco")
            nc.sync.dma_start(out=b1bc_or, in_=b1bc_in[1, :, :])
            b1bc = {True: b1bc_and, False: b1bc_or}

            w1_and = wpool.tile([128, KC1 * H], BF16, tag="w1a")
            nc.sync.dma_start(out=w1_and, in_=and_w1t[:, :])
            w2_and = wpool.tile([128, KC2 * H], BF16, tag="w2a")
            nc.sync.dma_start(out=w2_and, in_=and_w2t[:, :])
            w1_or = wpool.tile([128, KC1 * H], BF16, tag="w1o")
            w2_or = wpool.tile([128, KC2 * H], BF16, tag="w2o")
            w_not = wpool.tile([128, KC2 * H], BF16, tag="wn")
            w1 = {True: w1_and, False: w1_or}
            w2 = {True: w2_and, False: w2_or}

            hstate = {}

            def emit_l1(j):
                is_and = j < NAND
                t = inpool.tile([128, 2 * H], BF16, tag="pt_in")
                nc.sync.dma_start(out=t, in_=pairs_in[j, :, :])
                ph = psum.tile([128, H], F32, tag="ps_h")
                for k in range(KC1):
                    nc.tensor.matmul(
                        ph,
                        t[:, k * 128 : (k + 1) * 128],
                        w1[is_and][:, k * H : (k + 1) * H],
                        start=(k == 0),
                        stop=(k == KC1 - 1),
                    )
                htmp = hpool.tile([128, H], F32, tag="htmp_sb")
                nc.vector.scalar_tensor_tensor(
                    out=htmp,
                    in0=ph,
                    scalar=0.0,
                    in1=b1bc[is_and],
                    op0=mybir.AluOpType.bypass,
                    op1=mybir.AluOpType.add,
                )
                h = hpool.tile([128, H], BF16, tag="h_sb")
                nc.scalar.activation(h, htmp, mybir.ActivationFunctionType.Relu)
                hstate[j] = h

            htstate = {}

            def emit_trans(j):
                h = hstate.pop(j)
                pt = psum.tile([128, H], BF16, tag="ps_t")
                for c in range(KC2):
                    nc.tensor.transpose(
                        pt[:, c * 128 : (c + 1) * 128],
                        h[:, c * 128 : (c + 1) * 128],
                        ident,
                    )
                ht = htpool.tile([128, H], BF16, tag="ht_sb")
                nc.vector.tensor_copy(out=ht, in_=pt)
                htstate[j] = ht

            def emit_l2(j):
                is_and = j < NAND
                ht = htstate.pop(j)
                po = psum.tile([128, H], F32, tag="ps_o")
                for c in range(KC2):
                    nc.tensor.matmul(
                        po,
                        ht[:, c * 128 : (c + 1) * 128],
                        w2[is_and][:, c * H : (c + 1) * H],
                        start=(c == 0),
                        stop=(c == KC2 - 1),
                    )
                ot = opool.tile([128, H], F32, tag="o_sb")
                nc.vector.scalar_tensor_tensor(
                    out=ot,
                    in0=po,
                    scalar=0.0,
                    in1=b2bc[is_and],
                    op0=mybir.AluOpType.bypass,
                    op1=mybir.AluOpType.add,
                )
                nc.sync.dma_start(out=out_d[j, :, :], in_=ot)

            def emit_not(j):
                t = notpool.tile([128, H], BF16, tag="nt_in")
                nc.sync.dma_start(out=t, in_=nots_in[j, :, :])
                pn = psum.tile([128, H], F32, tag="ps_n")
                nc.tensor.matmul(pn, ones, b_not, start=True, stop=False)
                for c in range(KC2):
                    nc.tensor.matmul(
                        pn,
                        t[:, c * 128 : (c + 1) * 128],
                        w_not[:, c * H : (c + 1) * H],
                        start=False,
                        stop=(c == KC2 - 1),
                    )
                ot = opool.tile([128, H], F32, tag="o_sb")
                nc.scalar.activation(ot, pn, mybir.ActivationFunctionType.Tanh)
                nc.sync.dma_start(out=out_d[NPAIR + j, :, :], in_=ot)

            # 2-stage software skew: between the transposes of job j-1 and
            # their dependent L2 matmuls (which wait on the DVE copy of hT),
            # the PE runs L1(j) and L2(j-2) — no PE stall on ACT or DVE.
            emit_l1(0)
            emit_l1(1)
            emit_trans(0)
            for j in range(2, NPAIR):
                if j == 8:  # or-weights needed from job 16; queue their DMAs now
                    nc.sync.dma_start(out=w1_or, in_=or_w1t[:, :])
                if j == 12:
                    nc.sync.dma_start(out=w2_or, in_=or_w2t[:, :])
                    nc.sync.dma_start(out=w_not, in_=not_wt[:, :])
                emit_l1(j)
                emit_trans(j - 1)
                emit_l2(j - 2)
            emit_trans(NPAIR - 1)
            emit_l2(NPAIR - 2)
            emit_l2(NPAIR - 1)
            for j in range(NNOT):
                emit_not(j)

    nc.compile()
    return nc


def _get_nc():
    global _NC
    if _NC is None:
        _NC = _build_nc()
    return _NC


def _w1_tiles(W):  # [2H, H] -> [128, KC1*H], tile[p, k*H+n] = W[k*128+p, n]
    return np.ascontiguousarray(
        W.reshape(KC1, 128, H).transpose(1, 0, 2).reshape(128, KC1 * H).astype(BF16_NP)
    )


def _w2_tiles(W):  # [H, H] -> [128, KC2*H]
    return np.ascontiguousarray(
        W.reshape(KC2, 128, H).transpose(1, 0, 2).reshape(128, KC2 * H).astype(BF16_NP)
    )


def kernel(
    field_vectors,
    and_W1,
    and_b1,
    and_W2,
    and_b2,
    or_W1,
    or_b1,
    or_W2,
    or_b2,
    not_W,
    not_b,
    not_idx,
    and_pairs,
    or_pairs,
    leftover_idx,
):
    global LAST_RESULT
    fv = np.asarray(field_vectors, dtype=np.float32)
    and_W1 = np.asarray(and_W1, dtype=np.float32)
    and_W2 = np.asarray(and_W2, dtype=np.float32)
    or_W1 = np.asarray(or_W1, dtype=np.float32)
    or_W2 = np.asarray(or_W2, dtype=np.float32)
    not_W = np.asarray(not_W, dtype=np.float32)
    biases = np.stack(
        [
            np.asarray(and_b1, np.float32),
            np.asarray(or_b1, np.float32),
            np.asarray(and_b2, np.float32),
            np.asarray(or_b2, np.float32),
            np.asarray(not_b, np.float32),
        ]
    ).reshape(5, 1, H)
    not_idx = np.asarray(not_idx).astype(np.int64).ravel()
    and_pairs = np.asarray(and_pairs).astype(np.int64).reshape(NAND, 2)
    or_pairs = np.asarray(or_pairs).astype(np.int64).reshape(NOR, 2)
    leftover_idx = np.asarray(leftover_idx).astype(np.int64).ravel()

    not_set = set(not_idx.tolist())
    pair_fields = np.concatenate([and_pairs.ravel(), or_pairs.ravel()])  # 48 fields
    # The reference applies the not-transform before gathering pairs; with the
    # disjoint index structure used by this problem, pair fields are untouched.
    assert not (set(pair_fields.tolist()) & not_set), (
        "pair fields overlapping not_idx not supported"
    )

    shared = {
        "and_w1t": _w1_tiles(and_W1),
        "or_w1t": _w1_tiles(or_W1),
        "and_w2t": _w2_tiles(and_W2),
        "or_w2t": _w2_tiles(or_W2),
        "not_wt": _w2_tiles(not_W),
        "biases": np.ascontiguousarray(biases.astype(BF16_NP)),
        "b1bc": np.ascontiguousarray(
            np.stack(
                [
                    np.broadcast_to(np.asarray(and_b1, np.float32), (128, H)),
                    np.broadcast_to(np.asarray(or_b1, np.float32), (128, H)),
                ]
            )
        ),
        "b2bc": np.ascontiguousarray(
            np.stack(
                [
                    np.broadcast_to(np.asarray(and_b2, np.float32), (128, H)),
                    np.broadcast_to(np.asarray(or_b2, np.float32), (128, H)),
                ]
            )
        ),
        "ident": np.eye(128, dtype=np.float32).astype(BF16_NP),
    }

    in_maps = []
    for c in range(NCORES):
        fv_c = fv[c * BS : (c + 1) * BS]  # [128, 64, 512]
        G = fv_c[:, pair_fields, :]  # [128, 48, 512]
        G = (
            G.reshape(BS, NPAIR, 2, KC2, 128)
            .transpose(1, 4, 2, 3, 0)
            .reshape(NPAIR, 128, 2 * H)
        )
        N = fv_c[:, not_idx, :]  # [128, 8, 512]
        N = (
            N.reshape(BS, NNOT, KC2, 128)
            .transpose(1, 3, 2, 0)
            .reshape(NNOT, 128, H)
        )
        in_maps.append(
            {
                "pairs_in": np.ascontiguousarray(G.astype(BF16_NP)),
                "nots_in": np.ascontiguousarray(N.astype(BF16_NP)),
                **shared,
            }
        )

    nc = _get_nc()
    res = run_bass_kernel_spmd(nc, in_maps, core_ids=list(range(NCORES)), trace=TRACE)
    LAST_RESULT = res
    results = res.results if hasattr(res, "results") else res

    out = np.empty((B, NAND + NOR + len(leftover_idx), H), dtype=np.float32)
    not_slot = {int(f): j for j, f in enumerate(not_idx)}
    for c in range(NCORES):
        dev = results[c]["out"]  # [32, 128, 512]
        rows = slice(c * BS, (c + 1) * BS)
        out[rows, :NAND] = dev[:NAND].transpose(1, 0, 2)
        out[rows, NAND : NAND + NOR] = dev[NAND:NPAIR].transpose(1, 0, 2)
        for pos, f in enumerate(leftover_idx.tolist()):
            col = NAND + NOR + pos
            if f in not_slot:
                out[rows, col] = dev[NPAIR + not_slot[f]]
            else:
                out[rows, col] = fv[rows, f]
    return out



# revision 2
# speedup vs baseline: 1.3070x; 1.3070x over previous
"""Trainium2 Bass kernel for nn_LogicalOperatorFusion.

Semantics (matches the jax reference):
  fv = field_vectors                                  [B=1024, NF=64, H=512] f32
  fv[:, not_idx] = tanh(fv[:, not_idx] @ not_W + not_b)
  cat_and = concat(fv[:, and_pairs[:,0]], fv[:, and_pairs[:,1]], -1)   [B,16,1024]
  fused_and = relu(cat_and @ and_W1 + and_b1) @ and_W2 + and_b2        [B,16,512]
  cat_or  = ... same with or_pairs (8 pairs) ...                       [B,8,512]
  out = concat([fused_and, fused_or, fv[:, leftover_idx]], axis=1)     [B,40,512]

Strategy (v2):
  - Data parallel: batch sharded 8 ways (128 rows/core), weights replicated.
  - Weight-stationary matmuls throughout: lhsT = weight block [K=128, M=128],
    moving operand = activations^T packed 4 jobs wide (N = 4*128 rows = 512).
    L1 then produces the hidden layer ALREADY TRANSPOSED (hT chunks with H on
    partitions), so L2 consumes it directly -- no PE transposes at all.
  - With H on partitions, biases are per-partition: ACT applies
    relu(psum + b1) / (psum + b2) / tanh(psum + bn) in a single pass each.
    No bias matmuls, no extra DVE pass.
  - Jobs grouped 4-wide: groups 0-3 = and pairs, 4-5 = or pairs, 6-7 = nots.
    Per group: L1 = 32 MMs (4 m-chunks x 8 k-chunks, N=512), L2 = 16 MMs,
    NOT groups = 16 MMs. PSUM: 4 banks L1 + 4 banks L2, single-buffered with
    a strict L1(g),L2(g) interleave that keeps the PE stream dense.
  - Startup: w1_and + group-0 pairs stream in 8 k-chunk "step" DMAs so the
    first matmuls start ~3us in; ~29 garbage warmup MMs keep the PE busy from
    t~0 so the p-state ramp completes before real work begins.
  - Outputs leave as bf16 (tolerance 2e-2; bf16 adds ~3e-3), halving out DMA.
"""

import sys

for p in ("/opt/trn_rl_repo",):
    if p not in sys.path:
        sys.path.insert(0, p)

import numpy as np
import ml_dtypes

BF16_NP = ml_dtypes.bfloat16

import concourse.bacc as bacc
import concourse.mybir as mybir
from concourse.bass_utils import run_bass_kernel_spmd
from concourse.tile import TileContext

B, NF, H = 1024, 64, 512
NCORES = 8
BS = B // NCORES  # 128 rows per core
NAND, NOR, NNOT = 16, 8, 8
NPAIR = NAND + NOR  # 24
NPG = 6  # pair groups of 4 jobs
NNG = 2  # not groups of 4 jobs
F32 = mybir.dt.float32
BF16 = mybir.dt.bfloat16

TRACE = False  # test.py sets this for profiled runs
LAST_RESULT = None  # BassKernelResults of the last run (for test.py)

_NC = None  # cached traced Bass program

N_WARM = 29  # warmup matmuls covering the p-state ramp during input DMA


def _build_nc():
    """Trace the per-core Bass program (identical on all 8 cores)."""
    nc = bacc.Bacc("TRN2", target_bir_lowering=False)

    # step s: [w1a k-chunk s (4 m-blocks) | pairs g0 k-chunk s (4 jobs)]
    steps_d = nc.dram_tensor("steps", [8, 128, 1024], BF16, kind="ExternalInput")
    pairs_d = nc.dram_tensor("pairs", [NPG - 1, 128, 4096], BF16, kind="ExternalInput")
    w1o_d = nc.dram_tensor("w1o", [128, 4096], BF16, kind="ExternalInput")
    w2a_d = nc.dram_tensor("w2a", [128, 2048], BF16, kind="ExternalInput")
    w2o_d = nc.dram_tensor("w2o", [128, 2048], BF16, kind="ExternalInput")
    wn_d = nc.dram_tensor("wn", [128, 2048], BF16, kind="ExternalInput")
    nots_d = nc.dram_tensor("nots", [NNG, 128, 2048], BF16, kind="ExternalInput")
    # cols: b1a(0:4) b1o(4:8) b2a(8:12) b2o(12:16) bn(16:20); [p, m] = b[m*128+p]
    bias_d = nc.dram_tensor("biases", [128, 20], F32, kind="ExternalInput")
    out_d = nc.dram_tensor("out", [NPG + NNG, 128, 2048], BF16, kind="ExternalOutput")

    with TileContext(nc) as tc:
        with (
            tc.tile_pool(name="consts", bufs=1) as consts,
            tc.tile_pool(name="stepp", bufs=1) as stepp,
            tc.tile_pool(name="wpool", bufs=1) as wpool,
            tc.tile_pool(name="inpool", bufs=1) as inpool,
            tc.tile_pool(name="hpool", bufs=1) as hpool,
            tc.tile_pool(name="opool", bufs=2) as opool,
            tc.tile_pool(name="psumA", bufs=1, space="PSUM") as psumA,
            tc.tile_pool(name="psumB", bufs=1, space="PSUM") as psumB,
        ):
            # ---- input DMAs (SP engine, in consumption order) ----
            step_t = []
            for s in range(8):
                t = stepp.tile([128, 1024], BF16, tag=f"s{s}", name=f"step{s}")
                step_t.append(t)
            bias_t = consts.tile([128, 20], F32, tag="bias", name="bias_t")

            nc.sync.dma_start(out=step_t[0], in_=steps_d[0, :, :])
            nc.sync.dma_start(out=step_t[1], in_=steps_d[1, :, :])
            nc.sync.dma_start(out=bias_t, in_=bias_d[:, :])
            for s in range(2, 8):
                nc.sync.dma_start(out=step_t[s], in_=steps_d[s, :, :])

            w2a_t = wpool.tile([128, 2048], BF16, tag="w2a", name="w2a_t")
            nc.sync.dma_start(out=w2a_t, in_=w2a_d[:, :])
            pair_t = {}
            for g in range(1, 4):
                pair_t[g] = inpool.tile([128, 4096], BF16, tag=f"pg{g}", name=f"pg{g}")
                nc.sync.dma_start(out=pair_t[g], in_=pairs_d[g - 1, :, :])
            w1o_t = wpool.tile([128, 4096], BF16, tag="w1o", name="w1o_t")
            nc.sync.dma_start(out=w1o_t, in_=w1o_d[:, :])
            for g in range(4, 6):
                pair_t[g] = inpool.tile([128, 4096], BF16, tag=f"pg{g}", name=f"pg{g}")
                nc.sync.dma_start(out=pair_t[g], in_=pairs_d[g - 1, :, :])
            w2o_t = wpool.tile([128, 2048], BF16, tag="w2o", name="w2o_t")
            nc.sync.dma_start(out=w2o_t, in_=w2o_d[:, :])
            wn_t = wpool.tile([128, 2048], BF16, tag="wn", name="wn_t")
            nc.sync.dma_start(out=wn_t, in_=wn_d[:, :])
            not_t = []
            for n in range(NNG):
                t = inpool.tile([128, 2048], BF16, tag=f"ng{n}", name=f"ng{n}")
                not_t.append(t)
                nc.sync.dma_start(out=t, in_=nots_d[n, :, :])

            # ---- SBUF compute tiles ----
            hT = [
                hpool.tile([128, 512], BF16, tag=f"h{m}", name=f"hT{m}")
                for m in range(4)
            ]
            psA = [
                psumA.tile([128, 512], F32, tag=f"a{m}", name=f"psA{m}")
                for m in range(4)
            ]
            psB = [
                psumB.tile([128, 512], F32, tag=f"b{m}", name=f"psB{m}")
                for m in range(4)
            ]

            # ---- PE warmup: garbage matmuls during the input DMA latency ----
            warm_t = consts.tile([128, 128], BF16, tag="warm", name="warm_t")
            nc.vector.memset(warm_t, 0.0)
            for _ in range(N_WARM):
                nc.tensor.matmul(
                    psB[0][:, 0:128], warm_t, warm_t, start=True, stop=True
                )

            # weight-block slice helpers: lhsT [K=128, M=128]
            def w1_slice(g, kc, m):
                if g < 4:  # and: lives in the step tiles
                    return step_t[kc][:, m * 128 : (m + 1) * 128]
                return w1o_t[:, kc * 512 + m * 128 : kc * 512 + (m + 1) * 128]

            def pairs_rhs(g, kc):
                if g == 0:
                    return step_t[kc][:, 512:1024]
                return pair_t[g][:, kc * 512 : (kc + 1) * 512]

            def w2_slice(g, kc, m):
                w2 = w2a_t if g < 4 else w2o_t
                return w2[:, kc * 512 + m * 128 : kc * 512 + (m + 1) * 128]

            def b_col(group_kind, m):
                # group_kind: 0=b1 and, 1=b1 or, 2=b2 and, 3=b2 or, 4=bn
                c = group_kind * 4 + m
                return bias_t[:, c : c + 1]

            def l1_group(g, k_outer):
                b1k = 0 if g < 4 else 1
                order = (
                    [(kc, m) for kc in range(8) for m in range(4)]
                    if k_outer
                    else [(kc, m) for m in range(4) for kc in range(8)]
                )
                for kc, m in order:
                    nc.tensor.matmul(
                        psA[m],
                        w1_slice(g, kc, m),
                        pairs_rhs(g, kc),
                        start=(kc == 0),
                        stop=(kc == 7),
                    )
                    if kc == 7:
                        nc.scalar.activation(
                            hT[m],
                            psA[m],
                            mybir.ActivationFunctionType.Relu,
                            bias=b_col(b1k, m),
                        )

            def l2_group(g):
                b2k = 2 if g < 4 else 3
                out_t = opool.tile([128, 2048], BF16, tag="o", name=f"out{g}")
                for kc in range(4):  # k-outer: phase kc only needs hT[kc]
                    for m in range(4):
                        nc.tensor.matmul(
                            psB[m],
                            w2_slice(g, kc, m),
                            hT[kc],
                            start=(kc == 0),
                            stop=(kc == 3),
                        )
                for m in range(4):
                    nc.scalar.activation(
                        out_t[:, m * 512 : (m + 1) * 512],
                        psB[m],
                        mybir.ActivationFunctionType.Identity,
                        bias=b_col(b2k, m),
                    )
                nc.scalar.dma_start(out=out_d[g, :, :], in_=out_t)

            def not_group(n):
                out_t = opool.tile([128, 2048], BF16, tag="o", name=f"outn{n}")
                for m in range(4):  # m-outer: drain each chunk asap
                    for kc in range(4):
                        nc.tensor.matmul(
                            psA[m],
                            wn_t[:, kc * 512 + m * 128 : kc * 512 + (m + 1) * 128],
                            not_t[n][:, kc * 512 : (kc + 1) * 512],
                            start=(kc == 0),
                            stop=(kc == 3),
                        )
                    nc.scalar.activation(
                        out_t[:, m * 512 : (m + 1) * 512],
                        psA[m],
                        mybir.ActivationFunctionType.Tanh,
                        bias=b_col(4, m),
                    )
                    nc.scalar.dma_start(
                        out=out_d[NPG + n, :, m * 512 : (m + 1) * 512],
                        in_=out_t[:, m * 512 : (m + 1) * 512],
                    )

            l1_group(0, k_outer=True)  # paced by the step DMAs
            l2_group(0)
            for g in range(1, NPG):
                l1_group(g, k_outer=False)
                l2_group(g)
            not_group(0)
            not_group(1)

    nc.compile()
    return nc


def _get_nc():
    global _NC
    if _NC is None:
        _NC = _build_nc()
    return _NC


def _bf16(x):
    return np.ascontiguousarray(x.astype(BF16_NP))


def kernel(
    field_vectors,
    and_W1,
    and_b1,
    and_W2,
    and_b2,
    or_W1,
    or_b1,
    or_W2,
    or_b2,
    not_W,
    not_b,
    not_idx,
    and_pairs,
    or_pairs,
    leftover_idx,
):
    global LAST_RESULT
    fv = np.asarray(field_vectors, dtype=np.float32)
    and_W1 = np.asarray(and_W1, dtype=np.float32)
    and_W2 = np.asarray(and_W2, dtype=np.float32)
    or_W1 = np.asarray(or_W1, dtype=np.float32)
    or_W2 = np.asarray(or_W2, dtype=np.float32)
    not_W = np.asarray(not_W, dtype=np.float32)
    not_idx = np.asarray(not_idx).astype(np.int64).ravel()
    and_pairs = np.asarray(and_pairs).astype(np.int64).reshape(NAND, 2)
    or_pairs = np.asarray(or_pairs).astype(np.int64).reshape(NOR, 2)
    leftover_idx = np.asarray(leftover_idx).astype(np.int64).ravel()

    not_set = set(not_idx.tolist())
    pair_fields = np.concatenate([and_pairs.ravel(), or_pairs.ravel()])  # 48 fields
    # The reference applies the not-transform before gathering pairs; with the
    # disjoint index structure used by this problem, pair fields are untouched.
    assert not (set(pair_fields.tolist()) & not_set), (
        "pair fields overlapping not_idx not supported"
    )

    # ---- weight / bias packing (replicated across cores) ----
    # w1 block (kc, m): [p, n] = W1[kc*128+p, m*128+n]
    w1a_pack = and_W1.reshape(8, 128, 4, 128).transpose(1, 0, 2, 3)  # [p,kc,m,n]
    w1a_pack = w1a_pack.reshape(128, 8, 512)
    w1o_pack = or_W1.reshape(8, 128, 4, 128).transpose(1, 0, 2, 3).reshape(128, 4096)

    def _w2_pack(W):
        return W.reshape(4, 128, 4, 128).transpose(1, 0, 2, 3).reshape(128, 2048)

    biases = np.concatenate(
        [
            np.asarray(v, np.float32).reshape(4, 128).T
            for v in (and_b1, or_b1, and_b2, or_b2, not_b)
        ],
        axis=1,
    )  # [128, 20]

    shared = {
        "w1o": _bf16(w1o_pack),
        "w2a": _bf16(_w2_pack(and_W2)),
        "w2o": _bf16(_w2_pack(or_W2)),
        "wn": _bf16(_w2_pack(not_W)),
        "biases": np.ascontiguousarray(biases),
    }

    in_maps = []
    for c in range(NCORES):
        fv_c = fv[c * BS : (c + 1) * BS]  # [128, 64, 512]
        A = fv_c[:, pair_fields, :]  # [128, 48, 512]
        A6 = A.reshape(BS, 6, 4, 2, 4, 128)  # (r, g, jj, half, kc2, p)
        P = A6.transpose(1, 5, 3, 4, 2, 0).reshape(6, 128, 8, 512)  # (g,p,kc,(jj,r))
        steps = np.empty((8, 128, 1024), dtype=np.float32)
        steps[:, :, :512] = w1a_pack.transpose(1, 0, 2)  # (kc, p, (m,n))
        steps[:, :, 512:] = P[0].transpose(1, 0, 2)  # (kc, p, (jj,r))
        Nn = fv_c[:, not_idx, :]  # [128, 8, 512]
        N6 = Nn.reshape(BS, 2, 4, 4, 128)  # (r, n, jj, kc, p)
        NP_ = N6.transpose(1, 4, 3, 2, 0).reshape(2, 128, 2048)
        in_maps.append(
            {
                "steps": _bf16(steps),
                "pairs": _bf16(P[1:].reshape(5, 128, 4096)),
                "nots": _bf16(NP_),
                **shared,
            }
        )

    nc = _get_nc()
    res = run_bass_kernel_spmd(nc, in_maps, core_ids=list(range(NCORES)), trace=TRACE)
    LAST_RESULT = res
    results = res.results if hasattr(res, "results") else res

    out = np.empty((B, NAND + NOR + len(leftover_idx), H), dtype=np.float32)
    not_slot = {int(f): q for q, f in enumerate(not_idx)}
    for c in range(NCORES):
        dev = np.asarray(results[c]["out"]).astype(np.float32)  # [8, 128, 2048]
        rows = slice(c * BS, (c + 1) * BS)
        # group tile [p, (m2, jj, r)] -> (jj, r, (m2, p))
        jobs = dev.reshape(8, 128, 4, 4, 128).transpose(0, 3, 4, 2, 1)
        jobs = jobs.reshape(8, 4, BS, H)  # [group, jj, r, H]
        out[rows, :NPAIR] = (
            jobs[:NPG].reshape(NPAIR, BS, H).transpose(1, 0, 2)
        )
        for pos, f in enumerate(leftover_idx.tolist()):
            col = NPAIR + pos
            if f in not_slot:
                q = not_slot[f]
                out[rows, col] = jobs[NPG + q // 4, q % 4]
            else:
                out[rows, col] = fv[rows, f]
    return out


# revision 10
# speedup vs baseline: 1.3572x; 1.0384x over previous
"""Trainium2 Bass kernel for nn_LogicalOperatorFusion.

Semantics (matches the jax reference):
  fv = field_vectors                                  [B=1024, NF=64, H=512] f32
  fv[:, not_idx] = tanh(fv[:, not_idx] @ not_W + not_b)
  cat_and = concat(fv[:, and_pairs[:,0]], fv[:, and_pairs[:,1]], -1)   [B,16,1024]
  fused_and = relu(cat_and @ and_W1 + and_b1) @ and_W2 + and_b2        [B,16,512]
  cat_or  = ... same with or_pairs (8 pairs) ...                       [B,8,512]
  out = concat([fused_and, fused_or, fv[:, leftover_idx]], axis=1)     [B,40,512]

Strategy (v2):
  - Data parallel: batch sharded 8 ways (128 rows/core), weights replicated.
  - Weight-stationary matmuls throughout: lhsT = weight block [K=128, M=128],
    moving operand = activations^T packed 4 jobs wide (N = 4*128 rows = 512).
    L1 then produces the hidden layer ALREADY TRANSPOSED (hT chunks with H on
    partitions), so L2 consumes it directly -- no PE transposes at all.
  - With H on partitions, biases are per-partition: ACT applies
    relu(psum + b1) / (psum + b2) / tanh(psum + bn) in a single pass each.
    No bias matmuls, no extra DVE pass.
  - Jobs grouped 4-wide: groups 0-3 = and pairs, 4-5 = or pairs, 6-7 = nots.
    Per group: L1 = 32 MMs (4 m-chunks x 8 k-chunks, N=512), L2 = 16 MMs,
    NOT groups = 16 MMs. PSUM: 4 banks L1 + 4 banks L2, single-buffered with
    a strict L1(g),L2(g) interleave that keeps the PE stream dense.
  - Startup: w1_and + group-0 pairs stream in 8 k-chunk "step" DMAs so the
    first matmuls start ~3us in; ~29 garbage warmup MMs keep the PE busy from
    t~0 so the p-state ramp completes before real work begins.
  - Outputs leave as bf16 (tolerance 2e-2; bf16 adds ~3e-3), halving out DMA.
"""

import sys

for p in ("/opt/trn_rl_repo",):
    if p not in sys.path:
        sys.path.insert(0, p)

import numpy as np
import ml_dtypes

BF16_NP = ml_dtypes.bfloat16

import concourse.bacc as bacc
import concourse.mybir as mybir
from concourse.bass_utils import run_bass_kernel_spmd
from concourse.tile import TileContext

B, NF, H = 1024, 64, 512
NCORES = 8
BS = B // NCORES  # 128 rows per core
NAND, NOR, NNOT = 16, 8, 8
NPAIR = NAND + NOR  # 24
NPG = 6  # pair groups of 4 jobs
NNG = 2  # not groups of 4 jobs
F32 = mybir.dt.float32
BF16 = mybir.dt.bfloat16

TRACE = False  # test.py sets this for profiled runs
LAST_RESULT = None  # BassKernelResults of the last run (for test.py)

_NC = None  # cached traced Bass program

N_WARM = 29  # warmup matmuls covering the p-state ramp during input DMA


def _build_nc():
    """Trace the per-core Bass program (identical on all 8 cores)."""
    nc = bacc.Bacc("TRN2", target_bir_lowering=False)

    # step s: [w1a k-chunk s (4 m-blocks) | pairs g0 k-chunk s (4 jobs)]
    steps_d = nc.dram_tensor("steps", [8, 128, 1024], BF16, kind="ExternalInput")
    pairs_d = nc.dram_tensor("pairs", [NPG - 1, 128, 4096], BF16, kind="ExternalInput")
    w1o_d = nc.dram_tensor("w1o", [128, 4096], BF16, kind="ExternalInput")
    w2a_d = nc.dram_tensor("w2a", [128, 2048], BF16, kind="ExternalInput")
    w2o_d = nc.dram_tensor("w2o", [128, 2048], BF16, kind="ExternalInput")
    wn_d = nc.dram_tensor("wn", [128, 2048], BF16, kind="ExternalInput")
    nots_d = nc.dram_tensor("nots", [NNG, 128, 2048], BF16, kind="ExternalInput")
    # cols: b1a(0:4) b1o(4:8) b2a(8:12) b2o(12:16) bn(16:20); [p, m] = b[m*128+p]
    bias_d = nc.dram_tensor("biases", [128, 20], F32, kind="ExternalInput")
    out_d = nc.dram_tensor("out", [NPG + NNG, 128, 2048], BF16, kind="ExternalOutput")

    with TileContext(nc) as tc:
        with (
            tc.tile_pool(name="consts", bufs=1) as consts,
            tc.tile_pool(name="stepp", bufs=1) as stepp,
            tc.tile_pool(name="wpool", bufs=1) as wpool,
            tc.tile_pool(name="inpool", bufs=1) as inpool,
            tc.tile_pool(name="hpool", bufs=1) as hpool,
            tc.tile_pool(name="opool", bufs=3) as opool,
            tc.tile_pool(name="psumA", bufs=1, space="PSUM") as psumA,
            tc.tile_pool(name="psumB", bufs=1, space="PSUM") as psumB,
        ):
            # ---- input DMAs (SP engine, in consumption order) ----
            step_t = []
            for s in range(8):
                t = stepp.tile([128, 1024], BF16, tag=f"s{s}", name=f"step{s}")
                step_t.append(t)
            bias_t = consts.tile([128, 20], F32, tag="bias", name="bias_t")

            nc.sync.dma_start(out=step_t[0], in_=steps_d[0, :, :])
            nc.sync.dma_start(out=step_t[1], in_=steps_d[1, :, :])
            nc.sync.dma_start(out=bias_t, in_=bias_d[:, :])
            for s in range(2, 8):
                nc.sync.dma_start(out=step_t[s], in_=steps_d[s, :, :])

            w2a_t = wpool.tile([128, 2048], BF16, tag="w2a", name="w2a_t")
            nc.sync.dma_start(out=w2a_t, in_=w2a_d[:, :])
            pair_t = {}
            for g in range(1, 4):
                pair_t[g] = inpool.tile([128, 4096], BF16, tag=f"pg{g}", name=f"pg{g}")
                nc.sync.dma_start(out=pair_t[g], in_=pairs_d[g - 1, :, :])
            w1o_t = wpool.tile([128, 4096], BF16, tag="w1o", name="w1o_t")
            nc.sync.dma_start(out=w1o_t, in_=w1o_d[:, :])
            for g in range(4, 6):
                pair_t[g] = inpool.tile([128, 4096], BF16, tag=f"pg{g}", name=f"pg{g}")
                nc.sync.dma_start(out=pair_t[g], in_=pairs_d[g - 1, :, :])
            w2o_t = wpool.tile([128, 2048], BF16, tag="w2o", name="w2o_t")
            nc.sync.dma_start(out=w2o_t, in_=w2o_d[:, :])
            wn_t = wpool.tile([128, 2048], BF16, tag="wn", name="wn_t")
            nc.sync.dma_start(out=wn_t, in_=wn_d[:, :])
            not_t = []
            for n in range(NNG):
                t = inpool.tile([128, 2048], BF16, tag=f"ng{n}", name=f"ng{n}")
                not_t.append(t)
                nc.sync.dma_start(out=t, in_=nots_d[n, :, :])

            # ---- SBUF compute tiles ----
            hT = [
                hpool.tile([128, 512], BF16, tag=f"h{m}", name=f"hT{m}")
                for m in range(4)
            ]
            psA = [
                psumA.tile([128, 512], F32, tag=f"a{m}", name=f"psA{m}")
                for m in range(4)
            ]
            psB = [
                psumB.tile([128, 512], F32, tag=f"b{m}", name=f"psB{m}")
                for m in range(4)
            ]

            # ---- PE warmup: garbage matmuls during the input DMA latency ----
            warm_t = consts.tile([128, 128], BF16, tag="warm", name="warm_t")
            nc.gpsimd.memset(warm_t, 0.0)
            for _ in range(N_WARM):
                nc.tensor.matmul(
                    psB[0][:, 0:128], warm_t, warm_t, start=True, stop=True
                )

            # weight-block slice helpers: lhsT [K=128, M=128]
            def w1_slice(g, kc, m):
                if g < 4:  # and: lives in the step tiles
                    return step_t[kc][:, m * 128 : (m + 1) * 128]
                return w1o_t[:, kc * 512 + m * 128 : kc * 512 + (m + 1) * 128]

            def pairs_rhs(g, kc):
                if g == 0:
                    return step_t[kc][:, 512:1024]
                return pair_t[g][:, kc * 512 : (kc + 1) * 512]

            def w2_slice(g, kc, m):
                w2 = w2a_t if g < 4 else w2o_t
                return w2[:, kc * 512 + m * 128 : kc * 512 + (m + 1) * 128]

            def b_col(group_kind, m):
                # group_kind: 0=b1 and, 1=b1 or, 2=b2 and, 3=b2 or, 4=bn
                c = group_kind * 4 + m
                return bias_t[:, c : c + 1]

            def l1_group(g, k_outer):
                b1k = 0 if g < 4 else 1
                order = (
                    [(kc, m) for kc in range(8) for m in range(4)]
                    if k_outer
                    else [(kc, m) for m in range(4) for kc in range(8)]
                )
                for kc, m in order:
                    nc.tensor.matmul(
                        psA[m],
                        w1_slice(g, kc, m),
                        pairs_rhs(g, kc),
                        start=(kc == 0),
                        stop=(kc == 7),
                    )
                    if kc == 7:
                        nc.scalar.activation(
                            hT[m],
                            psA[m],
                            mybir.ActivationFunctionType.Relu,
                            bias=b_col(b1k, m),
                        )

            def l2_group(g):
                b2k = 2 if g < 4 else 3
                out_t = opool.tile([128, 2048], BF16, tag="o", name=f"out{g}")
                for kc in range(4):  # k-outer: phase kc only needs hT[kc]
                    for m in range(4):
                        nc.tensor.matmul(
                            psB[m],
                            w2_slice(g, kc, m),
                            hT[kc],
                            start=(kc == 0),
                            stop=(kc == 3),
                        )
                for m in range(4):
                    # bias-add drain on the otherwise-idle DVE, keeping the
                    # scalar engine free for relu/tanh
                    nc.vector.tensor_scalar_add(
                        out_t[:, m * 512 : (m + 1) * 512],
                        psB[m],
                        b_col(b2k, m),
                    )
                nc.sync.dma_start(out=out_d[g, :, :], in_=out_t)

            def not_group(n):
                out_t = opool.tile([128, 2048], BF16, tag="o", name=f"outn{n}")
                for m in range(4):  # m-outer: drain each chunk asap
                    # the very last chunk drains in halves so the tail chain
                    # (last MM -> ACT -> out DMA) is as short as possible
                    halves = (
                        [(0, 512, psA[m])]
                        if not (n == 1 and m == 3)
                        else [(0, 256, psA[m]), (256, 512, psB[3])]
                    )
                    for lo, hi, ps in halves:
                        for kc in range(4):
                            nc.tensor.matmul(
                                ps[:, lo:hi],
                                wn_t[:, kc * 512 + m * 128 : kc * 512 + (m + 1) * 128],
                                not_t[n][:, kc * 512 + lo : kc * 512 + hi],
                                start=(kc == 0),
                                stop=(kc == 3),
                            )
                        nc.scalar.activation(
                            out_t[:, m * 512 + lo : m * 512 + hi],
                            ps[:, lo:hi],
                            mybir.ActivationFunctionType.Tanh,
                            bias=b_col(4, m),
                        )
                        nc.sync.dma_start(
                            out=out_d[NPG + n, :, m * 512 + lo : m * 512 + hi],
                            in_=out_t[:, m * 512 + lo : m * 512 + hi],
                        )

            l1_group(0, k_outer=True)  # paced by the step DMAs
            l2_group(0)
            for g in range(1, NPG):
                l1_group(g, k_outer=False)
                l2_group(g)
            not_group(0)
            not_group(1)

    nc.compile()
    return nc


def _get_nc():
    global _NC
    if _NC is None:
        _NC = _build_nc()
    return _NC


def _bf16(x):
    return np.ascontiguousarray(x.astype(BF16_NP))


def kernel(
    field_vectors,
    and_W1,
    and_b1,
    and_W2,
    and_b2,
    or_W1,
    or_b1,
    or_W2,
    or_b2,
    not_W,
    not_b,
    not_idx,
    and_pairs,
    or_pairs,
    leftover_idx,
):
    global LAST_RESULT
    fv = np.asarray(field_vectors, dtype=np.float32)
    and_W1 = np.asarray(and_W1, dtype=np.float32)
    and_W2 = np.asarray(and_W2, dtype=np.float32)
    or_W1 = np.asarray(or_W1, dtype=np.float32)
    or_W2 = np.asarray(or_W2, dtype=np.float32)
    not_W = np.asarray(not_W, dtype=np.float32)
    not_idx = np.asarray(not_idx).astype(np.int64).ravel()
    and_pairs = np.asarray(and_pairs).astype(np.int64).reshape(NAND, 2)
    or_pairs = np.asarray(or_pairs).astype(np.int64).reshape(NOR, 2)
    leftover_idx = np.asarray(leftover_idx).astype(np.int64).ravel()

    not_set = set(not_idx.tolist())
    pair_fields = np.concatenate([and_pairs.ravel(), or_pairs.ravel()])  # 48 fields
    # The reference applies the not-transform before gathering pairs; with the
    # disjoint index structure used by this problem, pair fields are untouched.
    assert not (set(pair_fields.tolist()) & not_set), (
        "pair fields overlapping not_idx not supported"
    )

    # ---- weight / bias packing (replicated across cores) ----
    # w1 block (kc, m): [p, n] = W1[kc*128+p, m*128+n]
    w1a_pack = and_W1.reshape(8, 128, 4, 128).transpose(1, 0, 2, 3)  # [p,kc,m,n]
    w1a_pack = w1a_pack.reshape(128, 8, 512)
    w1o_pack = or_W1.reshape(8, 128, 4, 128).transpose(1, 0, 2, 3).reshape(128, 4096)

    def _w2_pack(W):
        return W.reshape(4, 128, 4, 128).transpose(1, 0, 2, 3).reshape(128, 2048)

    biases = np.concatenate(
        [
            np.asarray(v, np.float32).reshape(4, 128).T
            for v in (and_b1, or_b1, and_b2, or_b2, not_b)
        ],
        axis=1,
    )  # [128, 20]

    shared = {
        "w1o": _bf16(w1o_pack),
        "w2a": _bf16(_w2_pack(and_W2)),
        "w2o": _bf16(_w2_pack(or_W2)),
        "wn": _bf16(_w2_pack(not_W)),
        "biases": np.ascontiguousarray(biases),
    }

    in_maps = []
    for c in range(NCORES):
        fv_c = fv[c * BS : (c + 1) * BS]  # [128, 64, 512]
        A = fv_c[:, pair_fields, :]  # [128, 48, 512]
        A6 = A.reshape(BS, 6, 4, 2, 4, 128)  # (r, g, jj, half, kc2, p)
        P = A6.transpose(1, 5, 3, 4, 2, 0).reshape(6, 128, 8, 512)  # (g,p,kc,(jj,r))
        steps = np.empty((8, 128, 1024), dtype=np.float32)
        steps[:, :, :512] = w1a_pack.transpose(1, 0, 2)  # (kc, p, (m,n))
        steps[:, :, 512:] = P[0].transpose(1, 0, 2)  # (kc, p, (jj,r))
        Nn = fv_c[:, not_idx, :]  # [128, 8, 512]
        N6 = Nn.reshape(BS, 2, 4, 4, 128)  # (r, n, jj, kc, p)
        NP_ = N6.transpose(1, 4, 3, 2, 0).reshape(2, 128, 2048)
        in_maps.append(
            {
                "steps": _bf16(steps),
                "pairs": _bf16(P[1:].reshape(5, 128, 4096)),
                "nots": _bf16(NP_),
                **shared,
            }
        )

    nc = _get_nc()
    res = run_bass_kernel_spmd(nc, in_maps, core_ids=list(range(NCORES)), trace=TRACE)
    LAST_RESULT = res
    results = res.results if hasattr(res, "results") else res

    out = np.empty((B, NAND + NOR + len(leftover_idx), H), dtype=np.float32)
    not_slot = {int(f): q for q, f in enumerate(not_idx)}
    for c in range(NCORES):
        dev = np.asarray(results[c]["out"]).astype(np.float32)  # [8, 128, 2048]
        rows = slice(c * BS, (c + 1) * BS)
        # group tile [p, (m2, jj, r)] -> (jj, r, (m2, p))
        jobs = dev.reshape(8, 128, 4, 4, 128).transpose(0, 3, 4, 2, 1)
        jobs = jobs.reshape(8, 4, BS, H)  # [group, jj, r, H]
        out[rows, :NPAIR] = (
            jobs[:NPG].reshape(NPAIR, BS, H).transpose(1, 0, 2)
        )
        for pos, f in enumerate(leftover_idx.tolist()):
            col = NPAIR + pos
            if f in not_slot:
                q = not_slot[f]
                out[rows, col] = jobs[NPG + q // 4, q % 4]
            else:
                out[rows, col] = fv[rows, f]
    return out


# revision 24
# speedup vs baseline: 1.3646x; 1.0055x over previous
"""Trainium2 Bass kernel for nn_LogicalOperatorFusion.

Semantics (matches the jax reference):
  fv = field_vectors                                  [B=1024, NF=64, H=512] f32
  fv[:, not_idx] = tanh(fv[:, not_idx] @ not_W + not_b)
  cat_and = concat(fv[:, and_pairs[:,0]], fv[:, and_pairs[:,1]], -1)   [B,16,1024]
  fused_and = relu(cat_and @ and_W1 + and_b1) @ and_W2 + and_b2        [B,16,512]
  cat_or  = ... same with or_pairs (8 pairs) ...                       [B,8,512]
  out = concat([fused_and, fused_or, fv[:, leftover_idx]], axis=1)     [B,40,512]

Strategy (v2):
  - Data parallel: batch sharded 8 ways (128 rows/core), weights replicated.
  - Weight-stationary matmuls throughout: lhsT = weight block [K=128, M=128],
    moving operand = activations^T packed 4 jobs wide (N = 4*128 rows = 512).
    L1 then produces the hidden layer ALREADY TRANSPOSED (hT chunks with H on
    partitions), so L2 consumes it directly -- no PE transposes at all.
  - With H on partitions, biases are per-partition: ACT applies
    relu(psum + b1) / (psum + b2) / tanh(psum + bn) in a single pass each.
    No bias matmuls, no extra DVE pass.
  - Jobs grouped 4-wide: groups 0-3 = and pairs, 4-5 = or pairs, 6-7 = nots.
    Per group: L1 = 32 MMs (4 m-chunks x 8 k-chunks, N=512), L2 = 16 MMs,
    NOT groups = 16 MMs. PSUM: 4 banks L1 + 4 banks L2, single-buffered with
    a strict L1(g),L2(g) interleave that keeps the PE stream dense.
  - Startup: w1_and + group-0 pairs stream in 8 k-chunk "step" DMAs so the
    first matmuls start ~3us in; ~29 garbage warmup MMs keep the PE busy from
    t~0 so the p-state ramp completes before real work begins.
  - Outputs leave as bf16 (tolerance 2e-2; bf16 adds ~3e-3), halving out DMA.
"""

import sys

for p in ("/opt/trn_rl_repo",):
    if p not in sys.path:
        sys.path.insert(0, p)

import numpy as np
import ml_dtypes

BF16_NP = ml_dtypes.bfloat16

import concourse.bacc as bacc
import concourse.mybir as mybir
from concourse.bass_utils import run_bass_kernel_spmd
from concourse.tile import TileContext

B, NF, H = 1024, 64, 512
NCORES = 8
BS = B // NCORES  # 128 rows per core
NAND, NOR, NNOT = 16, 8, 8
NPAIR = NAND + NOR  # 24
NPG = 6  # pair groups of 4 jobs
NNG = 2  # not groups of 4 jobs
F32 = mybir.dt.float32
BF16 = mybir.dt.bfloat16

TRACE = False  # test.py sets this for profiled runs
LAST_RESULT = None  # BassKernelResults of the last run (for test.py)

_NC = None  # cached traced Bass program

import os

N_WARM = int(os.environ.get("N_WARM", "10"))  # warmup MMs covering the p-state ramp


def _build_nc():
    """Trace the per-core Bass program (identical on all 8 cores)."""
    nc = bacc.Bacc("TRN2", target_bir_lowering=False)

    # step s: [w1a k-chunk s (4 m-blocks) | pairs g0 k-chunk s (4 jobs)]
    steps_d = nc.dram_tensor("steps", [8, 128, 1024], BF16, kind="ExternalInput")
    pairs_d = nc.dram_tensor("pairs", [NPG - 1, 128, 4096], BF16, kind="ExternalInput")
    w1o_d = nc.dram_tensor("w1o", [128, 4096], BF16, kind="ExternalInput")
    w2a_d = nc.dram_tensor("w2a", [128, 2048], BF16, kind="ExternalInput")
    w2o_d = nc.dram_tensor("w2o", [128, 2048], BF16, kind="ExternalInput")
    wn_d = nc.dram_tensor("wn", [128, 2048], BF16, kind="ExternalInput")
    nots_d = nc.dram_tensor("nots", [NNG, 128, 2048], BF16, kind="ExternalInput")
    # cols: b1a(0:4) b1o(4:8) b2a(8:12) b2o(12:16) bn(16:20); [p, m] = b[m*128+p]
    bias_d = nc.dram_tensor("biases", [128, 20], F32, kind="ExternalInput")
    out_d = nc.dram_tensor("out", [NPG + NNG, 128, 2048], BF16, kind="ExternalOutput")

    with TileContext(nc) as tc:
        with (
            tc.tile_pool(name="consts", bufs=1) as consts,
            tc.tile_pool(name="stepp", bufs=1) as stepp,
            tc.tile_pool(name="wpool", bufs=1) as wpool,
            tc.tile_pool(name="inpool", bufs=1) as inpool,
            tc.tile_pool(name="hpool", bufs=1) as hpool,
            tc.tile_pool(name="opool", bufs=3) as opool,
            tc.tile_pool(name="psumA", bufs=1, space="PSUM") as psumA,
            tc.tile_pool(name="psumB", bufs=1, space="PSUM") as psumB,
        ):
            # ---- input DMAs (SP engine, in consumption order) ----
            step_t = []
            for s in range(8):
                t = stepp.tile([128, 1024], BF16, tag=f"s{s}", name=f"step{s}")
                step_t.append(t)
            bias_t = consts.tile([128, 20], F32, tag="bias", name="bias_t")

            # step 0 is split so the first matmul's operands (m0 weight block
            # + the group-0 pairs chunk, cols 0:640 -- see host packing) land
            # ~0.3us earlier than the rest
            nc.sync.dma_start(out=step_t[0], in_=steps_d[0, :, :])
            nc.sync.dma_start(out=step_t[1], in_=steps_d[1, :, :])
            nc.sync.dma_start(out=bias_t, in_=bias_d[:, :])
            for s in range(2, 8):
                nc.sync.dma_start(out=step_t[s], in_=steps_d[s, :, :])

            w2a_t = wpool.tile([128, 2048], BF16, tag="w2a", name="w2a_t")
            nc.sync.dma_start(out=w2a_t, in_=w2a_d[:, :])
            pair_t = {}
            for g in range(1, 4):
                pair_t[g] = inpool.tile([128, 4096], BF16, tag=f"pg{g}", name=f"pg{g}")
                nc.sync.dma_start(out=pair_t[g], in_=pairs_d[g - 1, :, :])
            w1o_t = wpool.tile([128, 4096], BF16, tag="w1o", name="w1o_t")
            nc.sync.dma_start(out=w1o_t, in_=w1o_d[:, :])
            for g in range(4, 6):
                pair_t[g] = inpool.tile([128, 4096], BF16, tag=f"pg{g}", name=f"pg{g}")
                nc.sync.dma_start(out=pair_t[g], in_=pairs_d[g - 1, :, :])
            w2o_t = wpool.tile([128, 2048], BF16, tag="w2o", name="w2o_t")
            nc.sync.dma_start(out=w2o_t, in_=w2o_d[:, :])
            wn_t = wpool.tile([128, 2048], BF16, tag="wn", name="wn_t")
            nc.sync.dma_start(out=wn_t, in_=wn_d[:, :])
            not_t = []
            for n in range(NNG):
                t = inpool.tile([128, 2048], BF16, tag=f"ng{n}", name=f"ng{n}")
                not_t.append(t)
                nc.sync.dma_start(out=t, in_=nots_d[n, :, :])

            # ---- SBUF compute tiles ----
            hT = [
                hpool.tile([128, 512], BF16, tag=f"h{m}", name=f"hT{m}")
                for m in range(4)
            ]
            psA = [
                psumA.tile([128, 512], F32, tag=f"a{m}", name=f"psA{m}")
                for m in range(4)
            ]
            psB = [
                psumB.tile([128, 512], F32, tag=f"b{m}", name=f"psB{m}")
                for m in range(4)
            ]

            # ---- PE warmup: garbage matmuls during the input DMA latency ----
            warm_t = consts.tile([128, 128], BF16, tag="warm", name="warm_t")
            nc.gpsimd.memset(warm_t, 0.0)
            for _ in range(N_WARM):
                nc.tensor.matmul(
                    psB[0][:, 0:128], warm_t, warm_t, start=True, stop=True
                )

            # weight-block slice helpers: lhsT [K=128, M=128]
            def w1_slice(g, kc, m):
                if g < 4:  # and: lives in the step tiles
                    return step_t[kc][:, m * 128 : (m + 1) * 128]
                return w1o_t[:, kc * 512 + m * 128 : kc * 512 + (m + 1) * 128]

            def pairs_rhs(g, kc):
                if g == 0:
                    return step_t[kc][:, 512:1024]
                return pair_t[g][:, kc * 512 : (kc + 1) * 512]

            def w2_slice(g, kc, m):
                w2 = w2a_t if g < 4 else w2o_t
                return w2[:, kc * 512 + m * 128 : kc * 512 + (m + 1) * 128]

            def b_col(group_kind, m):
                # group_kind: 0=b1 and, 1=b1 or, 2=b2 and, 3=b2 or, 4=bn
                c = group_kind * 4 + m
                return bias_t[:, c : c + 1]

            def l1_group(g, k_outer):
                b1k = 0 if g < 4 else 1
                order = (
                    [(kc, m) for kc in range(8) for m in range(4)]
                    if k_outer
                    else [(kc, m) for m in range(4) for kc in range(8)]
                )
                for kc, m in order:
                    nc.tensor.matmul(
                        psA[m],
                        w1_slice(g, kc, m),
                        pairs_rhs(g, kc),
                        start=(kc == 0),
                        stop=(kc == 7),
                    )
                    if kc == 7:
                        nc.scalar.activation(
                            hT[m],
                            psA[m],
                            mybir.ActivationFunctionType.Relu,
                            bias=b_col(b1k, m),
                        )

            def l2_group(g):
                b2k = 2 if g < 4 else 3
                out_t = opool.tile([128, 2048], BF16, tag="o", name=f"out{g}")
                for kc in range(4):  # k-outer: phase kc only needs hT[kc]
                    for m in range(4):
                        nc.tensor.matmul(
                            psB[m],
                            w2_slice(g, kc, m),
                            hT[kc],
                            start=(kc == 0),
                            stop=(kc == 3),
                        )
                for m in range(4):
                    # bias-add drain on the otherwise-idle DVE, keeping the
                    # scalar engine free for relu/tanh
                    nc.vector.tensor_scalar_add(
                        out_t[:, m * 512 : (m + 1) * 512],
                        psB[m],
                        b_col(b2k, m),
                    )
                nc.sync.dma_start(out=out_d[g, :, :], in_=out_t)

            def not_group(n):
                out_t = opool.tile([128, 2048], BF16, tag="o", name=f"outn{n}")
                for m in range(4):  # m-outer: drain each chunk asap
                    # the very last chunk drains in halves so the tail chain
                    # (last MM -> ACT -> out DMA) is as short as possible
                    halves = (
                        [(0, 512, psA[m])]
                        if not (n == 1 and m == 3)
                        else [(0, 256, psA[m]), (256, 512, psB[3])]
                    )
                    for lo, hi, ps in halves:
                        for kc in range(4):
                            nc.tensor.matmul(
                                ps[:, lo:hi],
                                wn_t[:, kc * 512 + m * 128 : kc * 512 + (m + 1) * 128],
                                not_t[n][:, kc * 512 + lo : kc * 512 + hi],
                                start=(kc == 0),
                                stop=(kc == 3),
                            )
                        nc.scalar.activation(
                            out_t[:, m * 512 + lo : m * 512 + hi],
                            ps[:, lo:hi],
                            mybir.ActivationFunctionType.Tanh,
                            bias=b_col(4, m),
                        )
                        nc.sync.dma_start(
                            out=out_d[NPG + n, :, m * 512 + lo : m * 512 + hi],
                            in_=out_t[:, m * 512 + lo : m * 512 + hi],
                        )

            l1_group(0, k_outer=True)  # paced by the step DMAs
            # two filler MMs bridge the wait for the first relu drain (hT[0])
            for _ in range(2):
                nc.tensor.matmul(
                    psB[0][:, 0:128], warm_t, warm_t, start=True, stop=True
                )
            l2_group(0)
            for g in range(1, NPG):
                l1_group(g, k_outer=False)
                l2_group(g)
            not_group(0)
            not_group(1)

    nc.compile()
    return nc


def _get_nc():
    global _NC
    if _NC is None:
        _NC = _build_nc()
    return _NC


def _bf16(x):
    return np.ascontiguousarray(x.astype(BF16_NP))


def kernel(
    field_vectors,
    and_W1,
    and_b1,
    and_W2,
    and_b2,
    or_W1,
    or_b1,
    or_W2,
    or_b2,
    not_W,
    not_b,
    not_idx,
    and_pairs,
    or_pairs,
    leftover_idx,
):
    global LAST_RESULT
    fv = np.asarray(field_vectors, dtype=np.float32)
    and_W1 = np.asarray(and_W1, dtype=np.float32)
    and_W2 = np.asarray(and_W2, dtype=np.float32)
    or_W1 = np.asarray(or_W1, dtype=np.float32)
    or_W2 = np.asarray(or_W2, dtype=np.float32)
    not_W = np.asarray(not_W, dtype=np.float32)
    not_idx = np.asarray(not_idx).astype(np.int64).ravel()
    and_pairs = np.asarray(and_pairs).astype(np.int64).reshape(NAND, 2)
    or_pairs = np.asarray(or_pairs).astype(np.int64).reshape(NOR, 2)
    leftover_idx = np.asarray(leftover_idx).astype(np.int64).ravel()

    not_set = set(not_idx.tolist())
    pair_fields = np.concatenate([and_pairs.ravel(), or_pairs.ravel()])  # 48 fields
    # The reference applies the not-transform before gathering pairs; with the
    # disjoint index structure used by this problem, pair fields are untouched.
    assert not (set(pair_fields.tolist()) & not_set), (
        "pair fields overlapping not_idx not supported"
    )

    # ---- weight / bias packing (replicated across cores) ----
    # w1 block (kc, m): [p, n] = W1[kc*128+p, m*128+n]
    w1a_pack = and_W1.reshape(8, 128, 4, 128).transpose(1, 0, 2, 3)  # [p,kc,m,n]
    w1a_pack = w1a_pack.reshape(128, 8, 512)
    w1o_pack = or_W1.reshape(8, 128, 4, 128).transpose(1, 0, 2, 3).reshape(128, 4096)

    def _w2_pack(W):
        return W.reshape(4, 128, 4, 128).transpose(1, 0, 2, 3).reshape(128, 2048)

    biases = np.concatenate(
        [
            np.asarray(v, np.float32).reshape(4, 128).T
            for v in (and_b1, or_b1, and_b2, or_b2, not_b)
        ],
        axis=1,
    )  # [128, 20]

    shared = {
        "w1o": _bf16(w1o_pack),
        "w2a": _bf16(_w2_pack(and_W2)),
        "w2o": _bf16(_w2_pack(or_W2)),
        "wn": _bf16(_w2_pack(not_W)),
        "biases": np.ascontiguousarray(biases),
    }

    in_maps = []
    for c in range(NCORES):
        fv_c = fv[c * BS : (c + 1) * BS]  # [128, 64, 512]
        A = fv_c[:, pair_fields, :]  # [128, 48, 512]
        A6 = A.reshape(BS, 6, 4, 2, 4, 128)  # (r, g, jj, half, kc2, p)
        P = A6.transpose(1, 5, 3, 4, 2, 0).reshape(6, 128, 8, 512)  # (g,p,kc,(jj,r))
        steps = np.empty((8, 128, 1024), dtype=np.float32)
        steps[:, :, :512] = w1a_pack.transpose(1, 0, 2)  # (kc, p, (m,n))
        steps[:, :, 512:] = P[0].transpose(1, 0, 2)  # (kc, p, (jj,r))
        Nn = fv_c[:, not_idx, :]  # [128, 8, 512]
        N6 = Nn.reshape(BS, 2, 4, 4, 128)  # (r, n, jj, kc, p)
        NP_ = N6.transpose(1, 4, 3, 2, 0).reshape(2, 128, 2048)
        in_maps.append(
            {
                "steps": _bf16(steps),
                "pairs": _bf16(P[1:].reshape(5, 128, 4096)),
                "nots": _bf16(NP_),
                **shared,
            }
        )

    nc = _get_nc()
    res = run_bass_kernel_spmd(nc, in_maps, core_ids=list(range(NCORES)), trace=TRACE)
    LAST_RESULT = res
    results = res.results if hasattr(res, "results") else res

    out = np.empty((B, NAND + NOR + len(leftover_idx), H), dtype=np.float32)
    not_slot = {int(f): q for q, f in enumerate(not_idx)}
    for c in range(NCORES):
        dev = np.asarray(results[c]["out"]).astype(np.float32)  # [8, 128, 2048]
        rows = slice(c * BS, (c + 1) * BS)
        # group tile [p, (m2, jj, r)] -> (jj, r, (m2, p))
        jobs = dev.reshape(8, 128, 4, 4, 128).transpose(0, 3, 4, 2, 1)
        jobs = jobs.reshape(8, 4, BS, H)  # [group, jj, r, H]
        out[rows, :NPAIR] = (
            jobs[:NPG].reshape(NPAIR, BS, H).transpose(1, 0, 2)
        )
        for pos, f in enumerate(leftover_idx.tolist()):
            col = NPAIR + pos
            if f in not_slot:
                q = not_slot[f]
                out[rows, col] = jobs[NPG + q // 4, q % 4]
            else:
                out[rows, col] = fv[rows, f]
    return out


# revision 30
# speedup vs baseline: 1.3700x; 1.0039x over previous
"""Trainium2 Bass kernel for nn_LogicalOperatorFusion.

Semantics (matches the jax reference):
  fv = field_vectors                                  [B=1024, NF=64, H=512] f32
  fv[:, not_idx] = tanh(fv[:, not_idx] @ not_W + not_b)
  cat_and = concat(fv[:, and_pairs[:,0]], fv[:, and_pairs[:,1]], -1)   [B,16,1024]
  fused_and = relu(cat_and @ and_W1 + and_b1) @ and_W2 + and_b2        [B,16,512]
  cat_or  = ... same with or_pairs (8 pairs) ...                       [B,8,512]
  out = concat([fused_and, fused_or, fv[:, leftover_idx]], axis=1)     [B,40,512]

Strategy (v2):
  - Data parallel: batch sharded 8 ways (128 rows/core), weights replicated.
  - Weight-stationary matmuls throughout: lhsT = weight block [K=128, M=128],
    moving operand = activations^T packed 4 jobs wide (N = 4*128 rows = 512).
    L1 then produces the hidden layer ALREADY TRANSPOSED (hT chunks with H on
    partitions), so L2 consumes it directly -- no PE transposes at all.
  - With H on partitions, biases are per-partition: ACT applies
    relu(psum + b1) / (psum + b2) / tanh(psum + bn) in a single pass each.
    No bias matmuls, no extra DVE pass.
  - Jobs grouped 4-wide: groups 0-3 = and pairs, 4-5 = or pairs, 6-7 = nots.
    Per group: L1 = 32 MMs (4 m-chunks x 8 k-chunks, N=512), L2 = 16 MMs,
    NOT groups = 16 MMs. PSUM: 4 banks L1 + 4 banks L2, single-buffered with
    a strict L1(g),L2(g) interleave that keeps the PE stream dense.
  - Startup: w1_and + group-0 pairs stream in 8 k-chunk "step" DMAs so the
    first matmuls start ~3us in; ~29 garbage warmup MMs keep the PE busy from
    t~0 so the p-state ramp completes before real work begins.
  - Outputs leave as bf16 (tolerance 2e-2; bf16 adds ~3e-3), halving out DMA.
"""

import sys

for p in ("/opt/trn_rl_repo",):
    if p not in sys.path:
        sys.path.insert(0, p)

import numpy as np
import ml_dtypes

BF16_NP = ml_dtypes.bfloat16

import concourse.bacc as bacc
import concourse.mybir as mybir
from concourse.bass_utils import run_bass_kernel_spmd
from concourse.tile import TileContext

B, NF, H = 1024, 64, 512
NCORES = 8
BS = B // NCORES  # 128 rows per core
NAND, NOR, NNOT = 16, 8, 8
NPAIR = NAND + NOR  # 24
NPG = 6  # pair groups of 4 jobs
NNG = 2  # not groups of 4 jobs
F32 = mybir.dt.float32
BF16 = mybir.dt.bfloat16

TRACE = False  # test.py sets this for profiled runs
LAST_RESULT = None  # BassKernelResults of the last run (for test.py)

_NC = None  # cached traced Bass program

import os

N_WARM = int(os.environ.get("N_WARM", "10"))  # warmup MMs covering the p-state ramp


def _build_nc():
    """Trace the per-core Bass program (identical on all 8 cores)."""
    nc = bacc.Bacc("TRN2", target_bir_lowering=False)

    # step s: [w1a k-chunk s (4 m-blocks) | pairs g0 k-chunk s (4 jobs)]
    steps_d = nc.dram_tensor("steps", [8, 128, 1024], BF16, kind="ExternalInput")
    pairs_d = nc.dram_tensor("pairs", [NPG - 1, 128, 4096], BF16, kind="ExternalInput")
    w1o_d = nc.dram_tensor("w1o", [128, 4096], BF16, kind="ExternalInput")
    w2a_d = nc.dram_tensor("w2a", [128, 2048], BF16, kind="ExternalInput")
    w2o_d = nc.dram_tensor("w2o", [128, 2048], BF16, kind="ExternalInput")
    wn_d = nc.dram_tensor("wn", [128, 2048], BF16, kind="ExternalInput")
    nots_d = nc.dram_tensor("nots", [NNG, 128, 2048], BF16, kind="ExternalInput")
    # cols: b1a(0:4) b1o(4:8) b2a(8:12) b2o(12:16) bn(16:20); [p, m] = b[m*128+p]
    bias_d = nc.dram_tensor("biases", [128, 20], F32, kind="ExternalInput")
    out_d = nc.dram_tensor("out", [NPG + NNG, 128, 2048], BF16, kind="ExternalOutput")

    with TileContext(nc) as tc:
        with (
            tc.tile_pool(name="consts", bufs=1) as consts,
            tc.tile_pool(name="stepp", bufs=1) as stepp,
            tc.tile_pool(name="wpool", bufs=1) as wpool,
            tc.tile_pool(name="inpool", bufs=1) as inpool,
            tc.tile_pool(name="hpool", bufs=1) as hpool,
            tc.tile_pool(name="opool", bufs=3) as opool,
            tc.tile_pool(name="psumA", bufs=1, space="PSUM") as psumA,
            tc.tile_pool(name="psumB", bufs=1, space="PSUM") as psumB,
        ):
            # ---- input DMAs (SP engine, in consumption order) ----
            step_t = []
            for s in range(8):
                t = stepp.tile([128, 1024], BF16, tag=f"s{s}", name=f"step{s}")
                step_t.append(t)
            bias_t = consts.tile([128, 20], F32, tag="bias", name="bias_t")

            # step 0 is split so the first matmul's operands (m0 weight block
            # + the group-0 pairs chunk, cols 0:640 -- see host packing) land
            # ~0.3us earlier than the rest
            nc.sync.dma_start(out=step_t[0], in_=steps_d[0, :, :])
            nc.sync.dma_start(out=step_t[1], in_=steps_d[1, :, :])
            nc.sync.dma_start(out=bias_t, in_=bias_d[:, :])
            for s in range(2, 8):
                nc.sync.dma_start(out=step_t[s], in_=steps_d[s, :, :])

            w2a_t = wpool.tile([128, 2048], BF16, tag="w2a", name="w2a_t")
            nc.sync.dma_start(out=w2a_t, in_=w2a_d[:, :])
            pair_t = {}
            for g in range(1, 4):
                pair_t[g] = inpool.tile([128, 4096], BF16, tag=f"pg{g}", name=f"pg{g}")
                nc.sync.dma_start(out=pair_t[g], in_=pairs_d[g - 1, :, :])
            w1o_t = wpool.tile([128, 4096], BF16, tag="w1o", name="w1o_t")
            nc.sync.dma_start(out=w1o_t, in_=w1o_d[:, :])
            for g in range(4, 6):
                pair_t[g] = inpool.tile([128, 4096], BF16, tag=f"pg{g}", name=f"pg{g}")
                nc.sync.dma_start(out=pair_t[g], in_=pairs_d[g - 1, :, :])
            w2o_t = wpool.tile([128, 2048], BF16, tag="w2o", name="w2o_t")
            nc.sync.dma_start(out=w2o_t, in_=w2o_d[:, :])
            wn_t = wpool.tile([128, 2048], BF16, tag="wn", name="wn_t")
            nc.sync.dma_start(out=wn_t, in_=wn_d[:, :])
            not_t = []
            for n in range(NNG):
                t = inpool.tile([128, 2048], BF16, tag=f"ng{n}", name=f"ng{n}")
                not_t.append(t)
                nc.sync.dma_start(out=t, in_=nots_d[n, :, :])

            # ---- SBUF compute tiles ----
            hT = [
                hpool.tile([128, 512], BF16, tag=f"h{m}", name=f"hT{m}")
                for m in range(4)
            ]
            psA = [
                psumA.tile([128, 512], F32, tag=f"a{m}", name=f"psA{m}")
                for m in range(4)
            ]
            psB = [
                psumB.tile([128, 512], F32, tag=f"b{m}", name=f"psB{m}")
                for m in range(4)
            ]

            # ---- PE warmup: garbage matmuls during the input DMA latency ----
            warm_t = consts.tile([128, 128], BF16, tag="warm", name="warm_t")
            nc.gpsimd.memset(warm_t, 0.0)
            for _ in range(N_WARM):
                nc.tensor.matmul(
                    psB[0][:, 0:128], warm_t, warm_t, start=True, stop=True
                )

            # weight-block slice helpers: lhsT [K=128, M=128]
            def w1_slice(g, kc, m):
                if g < 4:  # and: lives in the step tiles
                    return step_t[kc][:, m * 128 : (m + 1) * 128]
                return w1o_t[:, kc * 512 + m * 128 : kc * 512 + (m + 1) * 128]

            def pairs_rhs(g, kc):
                if g == 0:
                    return step_t[kc][:, 512:1024]
                return pair_t[g][:, kc * 512 : (kc + 1) * 512]

            def w2_slice(g, kc, m):
                w2 = w2a_t if g < 4 else w2o_t
                return w2[:, kc * 512 + m * 128 : kc * 512 + (m + 1) * 128]

            def b_col(group_kind, m):
                # group_kind: 0=b1 and, 1=b1 or, 2=b2 and, 3=b2 or, 4=bn
                c = group_kind * 4 + m
                return bias_t[:, c : c + 1]

            def l1_group(g, k_outer):
                b1k = 0 if g < 4 else 1
                order = (
                    [(kc, m) for kc in range(8) for m in range(4)]
                    if k_outer
                    else [(kc, m) for m in range(4) for kc in range(8)]
                )
                for kc, m in order:
                    nc.tensor.matmul(
                        psA[m],
                        w1_slice(g, kc, m),
                        pairs_rhs(g, kc),
                        start=(kc == 0),
                        stop=(kc == 7),
                    )
                    if kc == 7:
                        nc.scalar.activation(
                            hT[m],
                            psA[m],
                            mybir.ActivationFunctionType.Relu,
                            bias=b_col(b1k, m),
                        )

            def l2_group(g):
                b2k = 2 if g < 4 else 3
                out_t = opool.tile([128, 2048], BF16, tag="o", name=f"out{g}")
                for kc in range(4):  # k-outer: phase kc only needs hT[kc]
                    for m in range(4):
                        nc.tensor.matmul(
                            psB[m],
                            w2_slice(g, kc, m),
                            hT[kc],
                            start=(kc == 0),
                            stop=(kc == 3),
                        )
                for m in range(4):
                    # bias-add drain on the otherwise-idle DVE, keeping the
                    # scalar engine free for relu/tanh
                    nc.vector.tensor_scalar_add(
                        out_t[:, m * 512 : (m + 1) * 512],
                        psB[m],
                        b_col(b2k, m),
                    )
                nc.sync.dma_start(out=out_d[g, :, :], in_=out_t)

            def not_group(n):
                out_t = opool.tile([128, 2048], BF16, tag="o", name=f"outn{n}")
                for m in range(4):  # m-outer: drain each chunk asap
                    # the very last chunk drains in halves so the tail chain
                    # (last MM -> ACT -> out DMA) is as short as possible
                    halves = (
                        [(0, 512, psA[m])]
                        if not (n == 1 and m == 3)
                        else [(0, 256, psA[m]), (256, 512, psB[3])]
                    )
                    for lo, hi, ps in halves:
                        for kc in range(4):
                            nc.tensor.matmul(
                                ps[:, lo:hi],
                                wn_t[:, kc * 512 + m * 128 : kc * 512 + (m + 1) * 128],
                                not_t[n][:, kc * 512 + lo : kc * 512 + hi],
                                start=(kc == 0),
                                stop=(kc == 3),
                            )
                        nc.scalar.activation(
                            out_t[:, m * 512 + lo : m * 512 + hi],
                            ps[:, lo:hi],
                            mybir.ActivationFunctionType.Tanh,
                            bias=b_col(4, m),
                        )
                        # n1's m2 chunk drains via the idle Pool engine's
                        # SWDGE (no SP/HWDGE use), clearing the path for the
                        # final pieces on SP
                        use_pool = n == 1 and m == 2
                        eng = nc.gpsimd if use_pool else nc.sync
                        eng.dma_start(
                            out=out_d[NPG + n, :, m * 512 + lo : m * 512 + hi],
                            in_=out_t[:, m * 512 + lo : m * 512 + hi],
                        )

            l1_group(0, k_outer=True)  # paced by the step DMAs
            l2_group(0)
            for g in range(1, NPG):
                l1_group(g, k_outer=False)
                l2_group(g)
            not_group(0)
            not_group(1)

    nc.compile()
    return nc


def _get_nc():
    global _NC
    if _NC is None:
        _NC = _build_nc()
    return _NC


def _bf16(x):
    return np.ascontiguousarray(x.astype(BF16_NP))


def kernel(
    field_vectors,
    and_W1,
    and_b1,
    and_W2,
    and_b2,
    or_W1,
    or_b1,
    or_W2,
    or_b2,
    not_W,
    not_b,
    not_idx,
    and_pairs,
    or_pairs,
    leftover_idx,
):
    global LAST_RESULT
    fv = np.asarray(field_vectors, dtype=np.float32)
    and_W1 = np.asarray(and_W1, dtype=np.float32)
    and_W2 = np.asarray(and_W2, dtype=np.float32)
    or_W1 = np.asarray(or_W1, dtype=np.float32)
    or_W2 = np.asarray(or_W2, dtype=np.float32)
    not_W = np.asarray(not_W, dtype=np.float32)
    not_idx = np.asarray(not_idx).astype(np.int64).ravel()
    and_pairs = np.asarray(and_pairs).astype(np.int64).reshape(NAND, 2)
    or_pairs = np.asarray(or_pairs).astype(np.int64).reshape(NOR, 2)
    leftover_idx = np.asarray(leftover_idx).astype(np.int64).ravel()

    not_set = set(not_idx.tolist())
    pair_fields = np.concatenate([and_pairs.ravel(), or_pairs.ravel()])  # 48 fields
    # The reference applies the not-transform before gathering pairs; with the
    # disjoint index structure used by this problem, pair fields are untouched.
    assert not (set(pair_fields.tolist()) & not_set), (
        "pair fields overlapping not_idx not supported"
    )

    # ---- weight / bias packing (replicated across cores) ----
    # w1 block (kc, m): [p, n] = W1[kc*128+p, m*128+n]
    w1a_pack = and_W1.reshape(8, 128, 4, 128).transpose(1, 0, 2, 3)  # [p,kc,m,n]
    w1a_pack = w1a_pack.reshape(128, 8, 512)
    w1o_pack = or_W1.reshape(8, 128, 4, 128).transpose(1, 0, 2, 3).reshape(128, 4096)

    def _w2_pack(W):
        return W.reshape(4, 128, 4, 128).transpose(1, 0, 2, 3).reshape(128, 2048)

    biases = np.concatenate(
        [
            np.asarray(v, np.float32).reshape(4, 128).T
            for v in (and_b1, or_b1, and_b2, or_b2, not_b)
        ],
        axis=1,
    )  # [128, 20]

    shared = {
        "w1o": _bf16(w1o_pack),
        "w2a": _bf16(_w2_pack(and_W2)),
        "w2o": _bf16(_w2_pack(or_W2)),
        "wn": _bf16(_w2_pack(not_W)),
        "biases": np.ascontiguousarray(biases),
    }

    in_maps = []
    for c in range(NCORES):
        fv_c = fv[c * BS : (c + 1) * BS]  # [128, 64, 512]
        A = fv_c[:, pair_fields, :]  # [128, 48, 512]
        A6 = A.reshape(BS, 6, 4, 2, 4, 128)  # (r, g, jj, half, kc2, p)
        P = A6.transpose(1, 5, 3, 4, 2, 0).reshape(6, 128, 8, 512)  # (g,p,kc,(jj,r))
        steps = np.empty((8, 128, 1024), dtype=np.float32)
        steps[:, :, :512] = w1a_pack.transpose(1, 0, 2)  # (kc, p, (m,n))
        steps[:, :, 512:] = P[0].transpose(1, 0, 2)  # (kc, p, (jj,r))
        Nn = fv_c[:, not_idx, :]  # [128, 8, 512]
        N6 = Nn.reshape(BS, 2, 4, 4, 128)  # (r, n, jj, kc, p)
        NP_ = N6.transpose(1, 4, 3, 2, 0).reshape(2, 128, 2048)
        in_maps.append(
            {
                "steps": _bf16(steps),
                "pairs": _bf16(P[1:].reshape(5, 128, 4096)),
                "nots": _bf16(NP_),
                **shared,
            }
        )

    nc = _get_nc()
    res = run_bass_kernel_spmd(nc, in_maps, core_ids=list(range(NCORES)), trace=TRACE)
    LAST_RESULT = res
    results = res.results if hasattr(res, "results") else res

    out = np.empty((B, NAND + NOR + len(leftover_idx), H), dtype=np.float32)
    not_slot = {int(f): q for q, f in enumerate(not_idx)}
    for c in range(NCORES):
        dev = np.asarray(results[c]["out"]).astype(np.float32)  # [8, 128, 2048]
        rows = slice(c * BS, (c + 1) * BS)
        # group tile [p, (m2, jj, r)] -> (jj, r, (m2, p))
        jobs = dev.reshape(8, 128, 4, 4, 128).transpose(0, 3, 4, 2, 1)
        jobs = jobs.reshape(8, 4, BS, H)  # [group, jj, r, H]
        out[rows, :NPAIR] = (
            jobs[:NPG].reshape(NPAIR, BS, H).transpose(1, 0, 2)
        )
        for pos, f in enumerate(leftover_idx.tolist()):
            col = NPAIR + pos
            if f in not_slot:
                q = not_slot[f]
                out[rows, col] = jobs[NPG + q // 4, q % 4]
            else:
                out[rows, col] = fv[rows, f]
    return out
